# revision 14
# baseline (speedup 1.0000x reference)
"""BasicTransformerBlock Trainium2 Bass kernel (nn_BasicTransformerBlock_81570018885849).

Sharding: data-parallel, 2 frames/core x 8 cores; frame-0 K/V recomputed on
every core from a replicated h0 input (no collectives).

v2: K/V/O activations SBUF-resident per frame (no DRAM round trips for k, v,
o, of, o2); Q-side tensors stream through DRAM in 4-head chunks.  The dh=160
per-head split is handled with a packed B-residual layout: the 32-dim
residuals of 4 heads occupy one 128-partition group, so projection and
output-projection matmuls for the residuals run full-width.  Attention score
residual matmuls use row-group tile_position pairs (K=32 concurrency).
(DMA-xbar SBUF->SBUF transposes for LN were tried and are numerically
correct in CoreSim but RACE on hardware - token-tile-localized corruption
on random frames; LN transposes stay on the PE.)  Softmax denominators
ride in V slot 160; normalization via K=1 broadcast matmuls + DVE mults.  All
matmuls bf16 (fp32 PSUM); LN gains folded into weights host-side; all
additive biases are zero (asserted in prep_inputs).
"""
import numpy as np
import ml_dtypes

D, H, DH, DC, F, S, ENC, IP = 1280, 8, 160, 768, 16, 1024, 93, 16
FFD = 4 * D
NFF = FFD // 128     # 40
SCALE = DH ** -0.5
KT = D // 128        # 10
KC = DC // 128       # 6
TPF = S
NCORE, FPC = 8, 2
CH5 = [(c, 256) for c in range(0, 1280, 256)]

_perm = None
def perm():
    global _perm
    if _perm is None:
        p = []
        for t in range(H):
            p += list(range(t * DH, t * DH + 128))
        for h in range(H):
            p += list(range(h * DH + 128, h * DH + DH))
        _perm = np.array(p)
    return _perm


def _blocks_a(w):
    kt = w.shape[0] // 128
    wp = w[:, perm()]
    A = np.ascontiguousarray(wp[:, :1024].reshape(kt, 128, 8, 128).transpose(2, 1, 0, 3))
    B = np.ascontiguousarray(wp[:, 1024:].reshape(kt, 128, 256).transpose(1, 0, 2))
    return A, B


def _blob_b(w):
    kt = w.shape[0] // 128
    return np.ascontiguousarray(w.reshape(kt, 128, w.shape[1]).transpose(1, 0, 2))


def _wo_blobs4(w):
    wp = w[perm(), :]
    A = np.ascontiguousarray(wp[:1024].reshape(8, 128, D).transpose(1, 0, 2))
    B = np.ascontiguousarray(wp[1024:].reshape(2, 128, D).transpose(1, 0, 2))
    return A, B


_nc_cache = None

def build_nc():
    import concourse.mybir as mybir
    import concourse.tile as tile
    from concourse import bacc
    import contextlib

    F32, F32R, BF16 = mybir.dt.float32, mybir.dt.float32r, mybir.dt.bfloat16
    AF = mybir.ActivationFunctionType
    ALU = mybir.AluOpType

    nc = bacc.Bacc("TRN2", target_bir_lowering=False)

    def din(name, shape, dt):
        return nc.dram_tensor(name, list(shape), dt, kind="ExternalInput")

    i_h = din("h", (FPC * TPF, D), F32)
    i_h0 = din("h0", (TPF, D), F32)
    i_enc = din("enc", (FPC, ENC, DC), BF16)
    i_eyeb = din("eyeb", (128, 128), BF16)
    WA, WB = {}, {}
    for nm in ["q", "qf", "k", "q2"]:
        WA[nm] = din(f"w{nm}A", (8, 128, KT, 128), BF16)
        WB[nm] = din(f"w{nm}B", (128, KT, 256), BF16)
    for nm in ["k2", "k2i"]:
        WA[nm] = din(f"w{nm}A", (8, 128, KC, 128), BF16)
        WB[nm] = din(f"w{nm}B", (128, KC, 256), BF16)
    wv = din("wv", (128, KT, D), BF16)
    wv2 = din("wv2", (128, KC, D), BF16)
    wv2i = din("wv2i", (128, KC, D), BF16)
    WO = {}
    for nm in ["o", "of", "o2"]:
        WO[nm] = (din(f"w{nm}A", (128, 8, D), BF16), din(f"w{nm}B", (128, 2, D), BF16))
    wf1 = din("wf1", (2 * NFF, 128, KT, 128), BF16)
    wf2 = din("wf2", (128, NFF, D), BF16)
    o_h = nc.dram_tensor("h_out", [FPC * TPF, D], F32, kind="ExternalOutput")

    with tile.TileContext(nc) as tc:
        ctx = contextlib.ExitStack()
        with ctx:
            one = ctx.enter_context(tc.tile_pool(name="one", bufs=1))
            wkp = ctx.enter_context(tc.tile_pool(name="wkp", bufs=2))
            wk1 = ctx.enter_context(tc.tile_pool(name="wk1", bufs=2))
            wrk = ctx.enter_context(tc.tile_pool(name="wrk", bufs=2))
            psA = ctx.enter_context(tc.tile_pool(name="psA", bufs=4, space="PSUM"))
            psB = ctx.enter_context(tc.tile_pool(name="psB", bufs=2, space="PSUM"))
            psC = ctx.enter_context(tc.tile_pool(name="psC", bufs=2, space="PSUM"))
            drm = ctx.enter_context(tc.tile_pool(name="drm", bufs=1, space="DRAM"))

            # DRAM scratch: streamed q-side tensors + frame-0 K/V + residuals
            q_dA = drm.tile([128, 8, TPF], BF16);  q_dB = drm.tile([128, 2, TPF], BF16)
            qf_dA = drm.tile([128, 8, TPF], BF16); qf_dB = drm.tile([128, 2, TPF], BF16)
            q2_dA = drm.tile([128, 8, TPF], BF16); q2_dB = drm.tile([128, 2, TPF], BF16)
            k0A_d = drm.tile([128, 8, TPF], BF16)
            k0B_d = drm.tile([128, 2, TPF], BF16)
            v0_d = drm.tile([128, 8, 8, 161], BF16)   # [p, hh, tt, 161]
            h1_d = drm.tile([TPF, D], F32)
            h2_d = drm.tile([FPC * TPF, D], F32)

            eyeb = one.tile([128, 128], BF16)
            nc.sync.dma_start(eyeb[:], i_eyeb[:])
            ones_f = one.tile([1, 128], F32)
            nc.vector.memset(ones_f, 1.0)
            ones_r = ones_f[:].bitcast(F32R)
            ones_cb = one.tile([128, 1], BF16)
            nc.vector.memset(ones_cb, 1.0)
            eps = one.tile([128, 1], F32)
            nc.vector.memset(eps, 1e-5)

            nT = one.tile([128, KT, TPF], BF16, tag="nT")
            encT = one.tile([128, KC, 93], BF16, tag="encT")
            k2A = one.tile([128, 8, 93], BF16, tag="k2A")
            k2B4 = one.tile([128, 2, 93], BF16, tag="k2B4")
            v2t = one.tile([77, 8, 160], BF16, tag="v2t")
            v2i = one.tile([16, 8, 160], BF16, tag="v2i")

            FA = [None]
            qctr = [0]
            def dq():
                qctr[0] ^= 1
                return nc.sync if qctr[0] else nc.scalar

            # ---------- helpers ----------
            def ln_to_T(src_rows, ntt):
                for tt in range(ntt):
                    ht = wrk.tile([128, D], F32, tag="lnh")
                    dq().dma_start(ht[:], src_rows(tt))
                    st = wrk.tile([128, 5, 6], F32, tag="lns")
                    hr = ht[:].rearrange("p (n s) -> p n s", s=256)
                    for i in range(5):
                        nc.vector.bn_stats(st[:, i], hr[:, i])
                    mv = wrk.tile([128, 2], F32, tag="lnm")
                    nc.vector.bn_aggr(mv[:], st[:])
                    rs = wrk.tile([128, 1], F32, tag="lnr")
                    nc.scalar.activation(rs[:], mv[:, 1:2], AF.Sqrt, bias=eps[:])
                    nc.vector.reciprocal(rs[:], rs[:])
                    xh = wrk.tile([128, D], BF16, tag="lnx")
                    nc.vector.tensor_scalar(
                        xh[:], ht[:], scalar1=mv[:, 0:1], scalar2=rs[:],
                        op0=ALU.subtract, op1=ALU.mult)
                    for dt in range(KT):
                        pt = psA.tile([128, 128], BF16, tag="mm", name="pt_tr")
                        nc.tensor.transpose(pt[:], xh[:, 128 * dt:128 * dt + 128], eyeb[:])
                        nc.any.tensor_copy(nT[:, dt, 128 * tt:128 * tt + 128], pt[:])

            def proj_a(wAd, wBd, sinkA, sinkB):
                for t in range(8):
                    wt = wkp.tile([128, KT, 128], BF16, tag="wA", name="wt_a")
                    dq().dma_start(wt[:], wAd[t])
                    for c in range(2):
                        cs = slice(512 * c, 512 * c + 512)
                        p = psA.tile([128, 512], F32, tag="mm", name="p_a")
                        for dt in range(KT):
                            nc.tensor.matmul(p[:], wt[:, dt], nT[:, dt, cs],
                                             start=(dt == 0), stop=(dt == KT - 1))
                        sinkA(t, c, cs, p)
                wb = wkp.tile([128, KT, 256], BF16, tag="wBv", name="wb_a")
                dq().dma_start(wb[:], wBd[:])
                for g in range(2):
                    for c in range(2):
                        cs = slice(512 * c, 512 * c + 512)
                        p = psA.tile([128, 512], F32, tag="mm", name="p_b")
                        for dt in range(KT):
                            nc.tensor.matmul(p[:], wb[:, dt, 128 * g:128 * g + 128],
                                             nT[:, dt, cs], start=(dt == 0), stop=(dt == KT - 1))
                        sinkB(g, c, cs, p)

            def sink_sb(dst):
                return lambda t, c, cs, p: nc.vector.tensor_copy(dst[:, t, cs], p[:])

            def sink_dr(dst):
                def s(t, c, cs, p):
                    ob = wrk.tile([128, 512], BF16, tag="prcp")
                    nc.vector.tensor_copy(ob[:], p[:])
                    dq().dma_start(dst[:, t, cs], ob[:])
                return s

            def proj_v(sink_pair):
                # sink_pair(hp, tt, p[, :320])
                for hp in range(4):
                    wt = wkp.tile([128, KT, 320], BF16, tag="wBv", name="wt_v")
                    dq().dma_start(wt[:], wv[:, :, 320 * hp:320 * hp + 320])
                    for tt in range(8):
                        p = psA.tile([128, 512], F32, tag="mm", name="p_v")[:, 0:320]
                        for dt in range(KT):
                            nc.tensor.matmul(p, nT[:, dt, 128 * tt:128 * tt + 128],
                                             wt[:, dt], start=(dt == 0), stop=(dt == KT - 1))
                        sink_pair(hp, tt, p)

            # attention: q-side streamed per (g4, c) from DRAM; k/v via accessors.
            # Heads processed in pairs (g4 in 0..3 -> heads 2*g4, 2*g4+1).
            def attention(qAd, qBd, kA_at, kB_at, v_at, oA, oB4):
                for g4 in range(4):
                    g = g4 // 2
                    kA = kA_at(g4)         # [128, 2, 1024]
                    kB = kB_at(g4)         # [128, 1024] (packed group g)
                    vg = v_at(g4)          # [128, 2, 8, 161]
                    for c in range(2):
                        cs = slice(512 * c, 512 * c + 512)
                        qa = FA[0].tile([128, 2, 512], BF16, tag="qa", bufs=2)
                        dq().dma_start(qa[:], qAd[:, 2 * g4:2 * g4 + 2, cs])
                        qb = FA[0].tile([128, 512], BF16, tag="qb", bufs=2)
                        dq().dma_start(qb[:], qBd[:, g, cs])
                        o1s = [psA.tile([128, 512], F32, tag="mm", name=f"o1_{jj}")
                               for jj in range(2)]
                        o2s = [psC.tile([128, 512], F32, tag="o2", name=f"o2p{jj}")
                               for jj in range(2)]
                        pk = FA[0].tile([128, 2, 512], BF16, tag="pk", bufs=2)
                        for kj in range(8):
                            ks = slice(128 * kj, 128 * kj + 128)
                            sp = {}
                            for jj in range(2):
                                sp[jj] = psB.tile([128, 512], F32, tag="sp",
                                                  name=f"sp{jj}")
                                nc.tensor.matmul(sp[jj][:], kA[:, jj, ks], qa[:, jj, :],
                                                 start=True, stop=False)
                            for jj in range(2):
                                j = 2 * (g4 % 2) + jj
                                nc.tensor.matmul(sp[jj][:], kB[32 * j:32 * j + 32, ks],
                                                 qb[32 * j:32 * j + 32, :],
                                                 start=False, stop=True,
                                                 tile_position=(32 * j, 0))
                            for jj in range(2):
                                nc.scalar.activation(pk[:, jj, :], sp[jj][:], AF.Exp,
                                                     scale=float(SCALE))
                            for jj in range(2):
                                nc.tensor.matmul(o1s[jj][:], vg[:, jj, kj, 0:128],
                                                 pk[:, jj, :],
                                                 start=(kj == 0), stop=(kj == 7))
                                nc.tensor.matmul(o2s[jj][0:33, :], vg[:, jj, kj, 128:161],
                                                 pk[:, jj, :],
                                                 start=(kj == 0), stop=(kj == 7))
                        for jj in range(2):
                            h = 2 * g4 + jj
                            j = 2 * (g4 % 2) + jj
                            dn = FA[0].tile([1, 512], F32R, tag="dn", bufs=1)
                            nc.vector.tensor_copy(dn[:], o2s[jj][32:33, :])
                            with nc.allow_low_precision(reason="f32r recip == f32 bits"):
                                nc.vector.reciprocal(dn[:], dn[:])
                            rb = psB.tile([128, 512], F32, tag="sp", name="rb")
                            nc.tensor.matmul(rb[:], ones_r, dn[:], start=True, stop=True)
                            rbs = FA[0].tile([128, 512], F32R, tag="rbs", bufs=2)
                            nc.vector.tensor_copy(rbs[:], rb[:])
                            nc.vector.tensor_mul(oA[:, h, cs], o1s[jj][:], rbs[:])
                            nc.vector.tensor_mul(oB4[32 * j:32 * j + 32, g, cs],
                                                 o2s[jj][0:32, :], rbs[0:32, :])

            def wo_phase(sources, hsrc_rows, sink):
                # sources: list of (oA_ap, oB_ap, (wAd, wBd))
                nsrc = len(sources)
                for (c0, cw) in CH5:
                    wos = []
                    for si, (_, _, (wAd, wBd)) in enumerate(sources):
                        wa = wk1.tile([128, 8, 256], BF16, tag="woA", name=f"wa{si}")
                        dq().dma_start(wa[:], wAd[:, :, c0:c0 + cw])
                        wb = wk1.tile([128, 2, 256], BF16, tag="woB", name=f"wb{si}")
                        dq().dma_start(wb[:], wBd[:, :, c0:c0 + cw])
                        wos.append((wa, wb))
                    for tt in range(8):
                        ts_ = slice(128 * tt, 128 * tt + 128)
                        p = psA.tile([128, 512], F32, tag="mm", name="p_wo")[:, :cw]
                        first = True
                        for si, ((oA, oB, _), (wa, wb)) in enumerate(zip(sources, wos)):
                            for k in range(8):
                                nc.tensor.matmul(p, oA[:, k, ts_], wa[:, k, :],
                                                 start=first, stop=False)
                                first = False
                            for g in range(2):
                                nc.tensor.matmul(p, oB[:, g, ts_], wb[:, g, :],
                                                 start=False,
                                                 stop=(si == nsrc - 1 and g == 1))
                        hs = wrk.tile([128, 256], F32, tag="hres")
                        dq().dma_start(hs[:], hsrc_rows(tt, c0, cw))
                        sink(tt, c0, cw, p, hs)

            # ---------------- prologue: frame-0 K/V ----------------
            ln_to_T(lambda tt: i_h0[128 * tt:128 * tt + 128, :], 8)
            proj_a(WA["k"], WB["k"], sink_dr(k0A_d), sink_dr(k0B_d))

            def v0_sink(hp, tt, p):
                vtmp = wrk.tile([128, 2, 161], BF16, tag="vtmp")
                pr = p.rearrange("p (h d) -> p h d", h=2)
                nc.vector.tensor_copy(vtmp[:, :, 0:160], pr)
                nc.vector.memset(vtmp[:, :, 160:161], 1.0)
                dq().dma_start(v0_d[:, 2 * hp:2 * hp + 2, tt, :], vtmp[:])
            proj_v(v0_sink)

            # ---------------- frame loop ----------------
            for f in range(FPC):
                base = f * TPF
                with tc.tile_pool(name=f"fa{f}", bufs=3) as fa:
                    FA[0] = fa
                    ln_to_T(lambda tt: i_h[base + 128 * tt:base + 128 * tt + 128, :], 8)
                    proj_a(WA["qf"], WB["qf"], sink_dr(qf_dA), sink_dr(qf_dB))

                    kA_s = fa.tile([128, 8, TPF], BF16, tag="a16", name="kA")
                    kB_s = fa.tile([128, 2, TPF], BF16, tag="a4", name="kB")
                    proj_a(WA["k"], WB["k"], sink_sb(kA_s), sink_sb(kB_s))
                    v_s = fa.tile([128, 8, 8, 161], BF16, tag="v", name="v_s", bufs=1)

                    def v_sink(hp, tt, p):
                        pr = p.rearrange("p (h d) -> p h d", h=2)
                        nc.vector.tensor_copy(v_s[:, 2 * hp:2 * hp + 2, tt, 0:160], pr)
                    proj_v(v_sink)
                    nc.vector.memset(v_s[:, :, :, 160:161], 1.0)
                    proj_a(WA["q"], WB["q"], sink_dr(q_dA), sink_dr(q_dB))

                    # ---- attn1-ff: frame-0 K/V streamed from DRAM ----
                    ofA = fa.tile([128, 8, TPF], BF16, tag="a16", name="ofA")
                    ofB = fa.tile([128, 2, TPF], BF16, tag="a4", name="ofB")
                    def k0A_at(g4):
                        t = FA[0].tile([128, 2, TPF], BF16, tag="k0a", name=f"k0a{g4}", bufs=1)
                        dq().dma_start(t[:], k0A_d[:, 2 * g4:2 * g4 + 2, :])
                        return t[:]
                    def k0B_at(g4):
                        t = FA[0].tile([128, TPF], BF16, tag="k0b", name=f"k0b{g4}", bufs=1)
                        dq().dma_start(t[:], k0B_d[:, g4 // 2, :])
                        return t[:]
                    def v0_at(g4):
                        t = FA[0].tile([128, 2, 8, 161], BF16, tag="v0g", name=f"v0g{g4}", bufs=1)
                        dq().dma_start(t[:], v0_d[:, 2 * g4:2 * g4 + 2, :, :])
                        return t[:]
                    attention(qf_dA, qf_dB, k0A_at, k0B_at, v0_at, ofA[:], ofB[:])

                    # ---- attn1-self ----
                    oA = fa.tile([128, 8, TPF], BF16, tag="a16", name="oA")
                    oB = fa.tile([128, 2, TPF], BF16, tag="a4", name="oB")
                    attention(q_dA, q_dB,
                              lambda g4: kA_s[:, 2 * g4:2 * g4 + 2, :],
                              lambda g4: kB_s[:, g4 // 2, :],
                              lambda g4: v_s[:, 2 * g4:2 * g4 + 2, :, :],
                              oA[:], oB[:])

                    def sink_h1(tt, c0, cw, p, hs):
                        h1t = wrk.tile([128, 256], F32, tag="h1t")
                        nc.vector.tensor_add(h1t[:], p, hs[:])
                        dq().dma_start(h1_d[128 * tt:128 * tt + 128, c0:c0 + cw], h1t[:])
                    wo_phase([(oA[:], oB[:], WO["o"]), (ofA[:], ofB[:], WO["of"])],
                             lambda tt, c0, cw: i_h[base + 128 * tt:base + 128 * tt + 128,
                                                    c0:c0 + cw], sink_h1)

                    # ---- attn2 ----
                    ln_to_T(lambda tt: h1_d[128 * tt:128 * tt + 128, :], 8)
                    proj_a(WA["q2"], WB["q2"], sink_dr(q2_dA), sink_dr(q2_dB))

                    enc_s = wrk.tile([93, DC], BF16, tag="enc")
                    dq().dma_start(enc_s[:], i_enc[f])
                    for dc in range(KC):
                        pt = psA.tile([128, 128], BF16, tag="mm", name="pt_e")
                        nc.tensor.transpose(pt[:, 0:93], enc_s[:, 128 * dc:128 * dc + 128],
                                            eyeb[0:93, 0:93])
                        nc.vector.tensor_copy(encT[:, dc, :], pt[:, 0:93])

                    for t in range(8):
                        wt = wkp.tile([128, KC, 128], BF16, tag="wA", name="wt_k2")
                        dq().dma_start(wt[:], WA["k2"][t])
                        wti = wkp.tile([128, KC, 128], BF16, tag="wA", name="wt_k2i")
                        dq().dma_start(wti[:], WA["k2i"][t])
                        p = psA.tile([128, 512], F32, tag="mm", name="p_k2")
                        for dc in range(KC):
                            nc.tensor.matmul(p[:, 0:77], wt[:, dc], encT[:, dc, 0:77],
                                             start=(dc == 0), stop=(dc == KC - 1))
                        for dc in range(KC):
                            nc.tensor.matmul(p[:, 77:93], wti[:, dc], encT[:, dc, 77:93],
                                             start=(dc == 0), stop=(dc == KC - 1))
                        nc.vector.tensor_copy(k2A[:, t, :], p[:, 0:93])
                    wb2 = wkp.tile([128, KC, 256], BF16, tag="wBv", name="wb2")
                    dq().dma_start(wb2[:], WB["k2"][:])
                    wb2i = wkp.tile([128, KC, 256], BF16, tag="wBv", name="wb2i")
                    dq().dma_start(wb2i[:], WB["k2i"][:])
                    for g in range(2):
                        p = psA.tile([128, 512], F32, tag="mm", name="p_k2b")
                        for dc in range(KC):
                            nc.tensor.matmul(p[:, 0:77], wb2[:, dc, 128 * g:128 * g + 128],
                                             encT[:, dc, 0:77], start=(dc == 0), stop=(dc == KC - 1))
                        for dc in range(KC):
                            nc.tensor.matmul(p[:, 77:93], wb2i[:, dc, 128 * g:128 * g + 128],
                                             encT[:, dc, 77:93], start=(dc == 0), stop=(dc == KC - 1))
                        nc.vector.tensor_copy(k2B4[:, g, :], p[:, 0:93])

                    for (vsb, wsrc, np_, rng) in [(v2t, wv2, 77, slice(0, 77)),
                                                  (v2i, wv2i, 16, slice(77, 93))]:
                        for hp in range(4):
                            wt = wkp.tile([128, KC, 320], BF16, tag="wBv", name="wt_v2")
                            dq().dma_start(wt[:], wsrc[:, :, 320 * hp:320 * hp + 320])
                            p = psA.tile([128, 512], F32, tag="mm", name="p_v2")[0:np_, 0:320]
                            for dc in range(KC):
                                nc.tensor.matmul(p, encT[:, dc, rng], wt[:, dc],
                                                 start=(dc == 0), stop=(dc == KC - 1))
                            pr = p.rearrange("p (h d) -> p h d", h=2)
                            nc.vector.tensor_copy(vsb[:, 2 * hp:2 * hp + 2, :], pr)

                    o2A = fa.tile([128, 8, TPF], BF16, tag="a16", name="o2A")
                    o2B = fa.tile([128, 2, TPF], BF16, tag="a4", name="o2B")
                    for g in range(2):
                        for c in range(2):
                            cs = slice(512 * c, 512 * c + 512)
                            qa = fa.tile([128, 4, 512], BF16, tag="qa", bufs=2)
                            dq().dma_start(qa[:], q2_dA[:, 4 * g:4 * g + 4, cs])
                            qb = fa.tile([128, 512], BF16, tag="qb", bufs=2)
                            dq().dma_start(qb[:], q2_dB[:, g, cs])
                            for j in range(4):
                                h = 4 * g + j
                                spt = psB.tile([128, 512], F32, tag="sp", name="spt")[0:77, :]
                                nc.tensor.matmul(spt, k2A[:, h, 0:77], qa[:, j, :],
                                                 start=True, stop=False)
                                nc.tensor.matmul(spt, k2B4[32 * j:32 * j + 32, g, 0:77],
                                                 qb[32 * j:32 * j + 32, :],
                                                 start=False, stop=True,
                                                 tile_position=(32 * j, 0))
                                spi = psC.tile([128, 512], F32, tag="o2", name="spi")[0:16, :]
                                nc.tensor.matmul(spi, k2A[:, h, 77:93], qa[:, j, :],
                                                 start=True, stop=False)
                                nc.tensor.matmul(spi, k2B4[32 * j:32 * j + 32, g, 77:93],
                                                 qb[32 * j:32 * j + 32, :],
                                                 start=False, stop=True,
                                                 tile_position=(32 * j, 0))
                                pt2t = fa.tile([77, 512], BF16, tag="pt2t", bufs=2)
                                pt2i = fa.tile([16, 512], BF16, tag="pt2i", bufs=2)
                                nc.scalar.activation(pt2t[:], spt, AF.Exp, scale=float(SCALE))
                                nc.scalar.activation(pt2i[:], spi, AF.Exp, scale=float(SCALE))
                                dpt = psB.tile([128, 512], F32, tag="sp", name="dpt")[0:1, :]
                                nc.tensor.matmul(dpt, ones_cb[0:77, :], pt2t[:],
                                                 start=True, stop=True)
                                dpi = psC.tile([128, 512], F32, tag="o2", name="dpi")[0:1, :]
                                nc.tensor.matmul(dpi, ones_cb[0:16, :], pt2i[:],
                                                 start=True, stop=True)
                                dts = fa.tile([1, 512], F32R, tag="dn", bufs=1)
                                dis = fa.tile([1, 512], F32R, tag="dni", bufs=1)
                                nc.vector.tensor_copy(dts[:], dpt)
                                nc.vector.tensor_copy(dis[:], dpi)
                                with nc.allow_low_precision(reason="f32r recip == f32 bits"):
                                    nc.vector.reciprocal(dts[:], dts[:])
                                    nc.vector.reciprocal(dis[:], dis[:])
                                rbt = psB.tile([128, 512], F32, tag="sp", name="rbt")[0:77, :]
                                nc.tensor.matmul(rbt, ones_r[:, 0:77], dts[:], start=True, stop=True)
                                rbi = psC.tile([128, 512], F32, tag="o2", name="rbi")[0:16, :]
                                nc.tensor.matmul(rbi, ones_r[:, 0:16], dis[:], start=True, stop=True)
                                nc.vector.tensor_mul(pt2t[:], pt2t[:], rbt)
                                nc.vector.tensor_mul(pt2i[:], pt2i[:], rbi)
                                o1 = psA.tile([128, 512], F32, tag="mm", name="o1_2")
                                nc.tensor.matmul(o1[:], v2t[:, h, 0:128], pt2t[:], start=True, stop=False)
                                nc.tensor.matmul(o1[:], v2i[:, h, 0:128], pt2i[:], start=False, stop=True)
                                o2ps = psB.tile([128, 512], F32, tag="sp", name="o2_2")[0:32, :]
                                nc.tensor.matmul(o2ps, v2t[:, h, 128:160], pt2t[:], start=True, stop=False)
                                nc.tensor.matmul(o2ps, v2i[:, h, 128:160], pt2i[:], start=False, stop=True)
                                nc.vector.tensor_copy(o2A[:, h, cs], o1[:])
                                nc.vector.tensor_copy(o2B[32 * j:32 * j + 32, g, cs], o2ps)

                    def sink_h2(tt, c0, cw, p, hs):
                        h2t = wrk.tile([128, 256], F32, tag="h1t")
                        nc.vector.tensor_add(h2t[:], p, hs[:])
                        dq().dma_start(h2_d[base + 128 * tt:base + 128 * tt + 128,
                                            c0:c0 + cw], h2t[:])
                    wo_phase([(o2A[:], o2B[:], WO["o2"])],
                             lambda tt, c0, cw: h1_d[128 * tt:128 * tt + 128, c0:c0 + cw],
                             sink_h2)

            # ---------------- FF (4 chunks of 512 tokens) ----------------
            with tc.tile_pool(name="ffp", bufs=1) as ffp, \
                 tc.tile_pool(name="ffw", bufs=2) as ffw:
                for c4 in range(4):
                    base = c4 * 512
                    ln_to_T(lambda tt: h2_d[base + 128 * tt:base + 128 * tt + 128, :], 4)
                    innerT = ffp.tile([128, NFF, 512], BF16, tag="innerT", name="innerT")
                    for i in range(NFF):
                        wg = wkp.tile([128, KT, 128], BF16, tag="wA", name="wg")
                        dq().dma_start(wg[:], wf1[2 * i])
                        pg = psA.tile([128, 512], F32, tag="mm", name="pg")
                        for dt in range(KT):
                            nc.tensor.matmul(pg[:], wg[:, dt], nT[:, dt, 0:512],
                                             start=(dt == 0), stop=(dt == KT - 1))
                        gt = wrk.tile([128, 512], BF16, tag="gtmp")
                        nc.scalar.activation(gt[:], pg[:], AF.Gelu)
                        wa = wkp.tile([128, KT, 128], BF16, tag="wA", name="wa_f")
                        dq().dma_start(wa[:], wf1[2 * i + 1])
                        pa = psA.tile([128, 512], F32, tag="mm", name="pa")
                        for dt in range(KT):
                            nc.tensor.matmul(pa[:], wa[:, dt], nT[:, dt, 0:512],
                                             start=(dt == 0), stop=(dt == KT - 1))
                        nc.vector.tensor_mul(innerT[:, i, :], pa[:], gt[:])
                    for (c0, cw) in CH5:
                        w2c = ffw.tile([128, NFF, 256], BF16, tag="w2c")
                        dq().dma_start(w2c[:], wf2[:, :, c0:c0 + cw])
                        for tt in range(4):
                            p = psA.tile([128, 512], F32, tag="mm", name="pf2")[:, :cw]
                            for k in range(NFF):
                                nc.tensor.matmul(p, innerT[:, k, 128 * tt:128 * tt + 128],
                                                 w2c[:, k, :], start=(k == 0), stop=(k == NFF - 1))
                            hs = wrk.tile([128, 256], F32, tag="hres")
                            dq().dma_start(hs[:],
                                           h2_d[base + 128 * tt:base + 128 * tt + 128,
                                                c0:c0 + cw])
                            ho = wrk.tile([128, 256], F32, tag="h1t")
                            nc.vector.tensor_add(ho[:], p, hs[:])
                            dq().dma_start(o_h[base + 128 * tt:base + 128 * tt + 128,
                                               c0:c0 + cw], ho[:])

    nc.compile()
    return nc


def prep_inputs(inputs):
    gi = lambda k: np.asarray(inputs[k], np.float32)
    bf = lambda a: np.ascontiguousarray(a.astype(ml_dtypes.bfloat16))
    g1 = gi('ln1_g'); g2 = gi('ln2_g'); g3 = gi('ln3_g')
    for k in ['ln1_b', 'ln2_b', 'ln3_b', 'a1_wo_b', 'a1_wo_ff_b', 'a2_wo_b',
              'ff_b1', 'ff_b2']:
        assert np.abs(gi(k)).max() == 0.0, f"nonzero bias {k} unsupported"

    com = {}
    com['eyeb'] = bf(np.eye(128, dtype=np.float32))
    for nm, wkey, g in [("q", 'a1_wq', g1), ("qf", 'a1_wq_ff', g1),
                        ("k", 'a1_wk', g1), ("q2", 'a2_wq', g2)]:
        A, B = _blocks_a(g[:, None] * gi(wkey))
        com[f'w{nm}A'], com[f'w{nm}B'] = bf(A), bf(B)
    com['wv'] = bf(_blob_b(g1[:, None] * gi('a1_wv')))
    for nm, wkey in [("o", 'a1_wo'), ("of", 'a1_wo_ff'), ("o2", 'a2_wo')]:
        A, B = _wo_blobs4(gi(wkey))
        com[f'w{nm}A'], com[f'w{nm}B'] = bf(A), bf(B)
    for nm, wkey in [("k2", 'a2_wk'), ("k2i", 'a2_wk_ip')]:
        A, B = _blocks_a(gi(wkey))
        com[f'w{nm}A'], com[f'w{nm}B'] = bf(A), bf(B)
    com['wv2'] = bf(_blob_b(gi('a2_wv')))
    com['wv2i'] = bf(_blob_b(gi('a2_wv_ip')))
    w1 = g3[:, None] * gi('ff_w1')
    r = w1.reshape(KT, 128, 2 * NFF, 128).transpose(2, 1, 0, 3)
    order = []
    for i in range(NFF):
        order += [NFF + i, i]
    com['wf1'] = bf(r[order])
    com['wf2'] = bf(_blob_b(gi('ff_w2')))

    hs = gi('hidden_states')
    enc = gi('encoder_hidden_states')
    in_maps = []
    for c in range(NCORE):
        m = dict(com)
        m['h'] = np.ascontiguousarray(hs[2 * c:2 * c + 2].reshape(FPC * TPF, D))
        m['h0'] = np.ascontiguousarray(hs[0])
        m['enc'] = bf(enc[2 * c:2 * c + 2])
        in_maps.append(m)
    return in_maps


def kernel(**inputs):
    global _nc_cache
    from concourse.bass_utils import run_bass_kernel_spmd
    if _nc_cache is None:
        _nc_cache = build_nc()
    in_maps = prep_inputs(inputs)
    res = run_bass_kernel_spmd(_nc_cache, in_maps, core_ids=list(range(NCORE)))
    out = np.empty((F, S, D), np.float32)
    for c in range(NCORE):
        out[2 * c:2 * c + 2] = res.results[c]['h_out'].reshape(FPC, S, D)
    return out
